# revision 31
# baseline (speedup 1.0000x reference)
"""Trainium2 Bass kernel for the BDH fast-weight recurrent network.

Problem (see reference): for each batch element, a T=256-step recurrence with
  x_t   = L1norm(0.97*x_{t-1} + relu(v_t @ Dx^T))          (v_t = token_emb[idx_t])
  a*_t  = rho_{t-1} x_t ;  rho_t = 0.97*(rho_{t-1} + LN(v_t) x_t^T)
  y_t   = relu(LN(a*_t) @ Dy^T) * relu(x_t)
  out_t = LN(y_t @ E^T)

The kernel restructures this into feed-forward matmuls:
 - rho never materializes: a*_t = sum_{s<t} 0.97^{t-s} (x_s . x_t) LN(v_s)
   (decayed linear attention over the x sequence).
 - the x recurrence is linear given the per-step L1 scales S_t; since S_t ~ 100
   and eps=1e-6, S_t = sum(r_t) + 0.97 exactly in fp32, so X = G @ R with
   G[t,s] = 0.97^{t-s} / prod_{j=s..t} S_j.  G factors as
   gexp[s,t] * P_{s-1} / P_t with P_t = prod_{j<=t} (S_j/100) (range ~1, fp32
   safe) and gexp = host-precomputed exp part.  P is a prefix product done
   with a DVE scan; P_{s-1} comes from a shifted copy of the scan output (no
   reciprocal), 1/P returns to row layout via a partition-side reciprocal +
   PE transpose (fast) instead of a 128-element row reciprocal (940ns).
 - X carries a constant 2^8 factor (from the fp16-range shift in gexp).
 - x_t >= 0 so relu(x_t) = x_t and relu(y)*x = relu(y*x): the y-relu and the
   x multiply fuse into one DVE scalar_tensor_tensor pass (max then mult).
 - layernorms divide by (std + eps*2^k) directly, matching reference eps
   semantics exactly (no epsv folding).

Engine balance: the three 1M-element PSUM->SBUF passes per batch (R relu,
X^T cast, y-relu-mult) are split between the Scalar(ACT) and DVE engines;
a couple of SBUF-only multiplies ride on GpSimd.  Emission interleaves the
next batch's R matmuls and G chain into the current batch's serial LN/cast
regions so the PE queue never drains.

Sharding: data-parallel over batch, 4 sequences per NeuronCore x 8 cores,
no cross-core communication.
"""

import sys

if "/opt/trn_rl_repo" not in sys.path:
    sys.path.insert(0, "/opt/trn_rl_repo")

import numpy as np

import concourse.bass as bass
import concourse.bacc as bacc
import concourse.tile as tile
from concourse import mybir
from concourse.bass_utils import run_bass_kernel_spmd

AF = mybir.ActivationFunctionType
OP = mybir.AluOpType

N, D, V = 4096, 256, 32000
B, T = 32, 256
BL = 4              # batch per core
NCORES = 8
XD = 0.97           # x decay
UD = 0.97           # rho decay
EPS = 1e-6
MU = float(np.log(100.0))
LNXD = float(np.log(XD))

F32 = mybir.dt.float32
F16 = mybir.dt.float16
MODE = "f16"
MODE_DT = {"f32": mybir.dt.float32, "f32r": mybir.dt.float32r,
           "f16": mybir.dt.float16, "fp8": mybir.dt.float16}
MM_DT = MODE_DT[MODE]
GT_LOG_SCALE = 8.0 * float(np.log(2.0))   # store GT * 2^8 (fp16 underflow guard)
EPS_A = EPS * float(2.0 ** 16)            # a* psum carries 2^16 (= 2*2^8)
EPS_V = EPS * float(2.0 ** 8)             # v* psum carries 2^8 (x~ has 2^8)

NT = N // 128       # 32 n tiles
TT = T // 128       # 2 t tiles
DT = D // 128       # 2 d tiles
DDOF = float(D) / (D - 1)

# engine split knobs (chunk index -> ACT engine; rest DVE)
import os as _os

def _knob(name, default):
    v = _os.environ.get(name)
    if v is None:
        return default
    return tuple(int(x) for x in v.split(",") if x != "")

ACT_R = _knob("K_ACT_R", (0, 2, 4, 6))    # of 8 R-relu units [128,1024]/batch
ACT_XT = _knob("K_ACT_XT", (0, 2, 4, 6))  # of 8 XT-cast units per batch
ACT_Y = _knob("K_ACT_Y", (1, 3, 5))       # of 8 y units: ACT relu + mult; rest DVE STT
POOL_Y = _knob("K_POOL_Y", (3,))          # subset of ACT_Y whose mult goes to gpsimd


def _host_consts():
    """Constant tensors shipped to every core (computed in float64, cast f32)."""
    si = np.arange(T, dtype=np.float64)[:, None]
    ti = np.arange(T, dtype=np.float64)[None, :]
    k = ti - si
    kconst = np.where(k >= 0, k * LNXD - (k + 1) * MU + GT_LOG_SCALE, -np.inf)
    gexp = np.exp(kconst).astype(np.float32)          # banded: underflow -> 0
    gexp = gexp.reshape(TT, 128, T).transpose(1, 0, 2)
    decayT = np.where(k > 0, UD ** np.maximum(k, 0.0), 0.0)
    decayT = decayT.astype(np.float32).reshape(TT, 128, T).transpose(1, 0, 2)
    svb = np.full((T,), XD, np.float32)
    svb[0] = 0.0
    svb = svb.reshape(TT, 128).T.copy()
    return {
        "gexp": np.ascontiguousarray(gexp),       # (128, TT, T)
        "decayT": np.ascontiguousarray(decayT),   # (128, TT, T)
        "svb": np.ascontiguousarray(svb),         # (128, TT)
    }


def build_nc(mm_dt=MM_DT, dbg=False, dbg_keys=None):
    nc = bacc.Bacc("TRN2", target_bir_lowering=False, debug=False)


    ud_d = nc.dram_tensor("uln", [BL, 128, TT, D], F16, kind="ExternalInput").ap()
    vptd = nc.dram_tensor("vprevT", [BL, 128, DT, T], F16, kind="ExternalInput").ap()
    dxt_d = nc.dram_tensor("dxt", [D, N], mm_dt, kind="ExternalInput").ap()
    dyt_d = nc.dram_tensor("dyt", [D, N], mm_dt, kind="ExternalInput").ap()
    et_d = nc.dram_tensor("et", [N, D], mm_dt, kind="ExternalInput").ap()
    gexp_d = nc.dram_tensor("gexp", [128, TT, T], F32, kind="ExternalInput").ap()
    decayT_d = nc.dram_tensor("decayT", [128, TT, T], F32, kind="ExternalInput").ap()
    svb_d = nc.dram_tensor("svb", [128, TT], F32, kind="ExternalInput").ap()
    identh_d = nc.dram_tensor("identh", [128, 128], F16, kind="ExternalInput").ap()
    esel_d = nc.dram_tensor("esel", [2, TT, 128], F16, kind="ExternalInput").ap()
    out_d = nc.dram_tensor("out", [BL, T, D], F32, kind="ExternalOutput").ap()

    with tile.TileContext(nc) as tc:
        with (
            tc.tile_pool(name="consts", bufs=1) as consts,
            tc.tile_pool(name="big", bufs=2) as big,
            tc.tile_pool(name="mid", bufs=2) as mid,
            tc.tile_pool(name="tiny", bufs=10) as tiny,
            tc.tile_pool(name="scratch", bufs=6) as scratch,
            tc.tile_pool(name="vpool", bufs=3) as vpool,
            tc.tile_pool(name="psX", bufs=2, space="PSUM") as psX,
            tc.tile_pool(name="psA", bufs=3, space="PSUM") as psA,
            tc.tile_pool(name="psS", bufs=1, space="PSUM") as psS,
        ):
            # ---- embedding rows are gathered host-side (idx known there) ----
            vprevs = {}

            def stage_gather(b, eng=None):
                U = vpool.tile([128, TT, D], F16, tag="U")
                vprevT = vpool.tile([128, DT, T], F16, tag="vprevT")
                vprevs[b] = (U, vprevT)
                e = eng if eng is not None else nc.sync
                e.dma_start(out=vprevT[:], in_=vptd[b])
                e.dma_start(out=U[:], in_=ud_d[b])

            dxt = consts.tile([128, DT, N], mm_dt)
            dxt_src = dxt_d.rearrange("(k p) n -> p k n", p=128)
            # critical prologue transfers: gather(0) dispatches on the (idle)
            # ACT queue in parallel with the first dxt chunks on SP
            stage_gather(0, eng=nc.scalar)
            for kd in range(DT):
                nc.sync.dma_start(out=dxt[:, kd, 0:512],
                                  in_=dxt_src[:, kd, 0:512])
            for kd in range(DT):
                nc.sync.dma_start(out=dxt[:, kd, 512:1024],
                                  in_=dxt_src[:, kd, 512:1024])
            for q in range(1, 4):
                for kd in range(DT):
                    nc.sync.dma_start(
                        out=dxt[:, kd, q * 1024 : (q + 1) * 1024],
                        in_=dxt_src[:, kd, q * 1024 : (q + 1) * 1024])
            stage_gather(1)
            identh = consts.tile([128, 128], F16)
            nc.sync.dma_start(out=identh[:], in_=identh_d[:])
            svb = consts.tile([128, TT], F32)
            nc.sync.dma_start(out=svb[:], in_=svb_d[:])
            gexp = consts.tile([128, TT, T], F32)
            nc.sync.dma_start(out=gexp[:], in_=gexp_d[:])
            esel = consts.tile([2, TT, 128], F16)
            nc.sync.dma_start(out=esel[:], in_=esel_d[:])

            decayT = consts.tile([128, TT, T], F32)
            dyt = consts.tile([128, DT, N], mm_dt)
            et = consts.tile([128, NT, D], mm_dt)

            def emit_late_consts():
                nc.sync.dma_start(out=decayT[:], in_=decayT_d[:])
                dyt_src = dyt_d.rearrange("(k p) n -> p k n", p=128)
                for kd in range(DT):
                    nc.sync.dma_start(out=dyt[:, kd, :], in_=dyt_src[:, kd, :])
                et_src = et_d.rearrange("(k p) d -> p k d", p=128)
                for kq in range(4):
                    nc.sync.dma_start(out=et[:, kq * 8 : (kq + 1) * 8, :],
                                      in_=et_src[:, kq * 8 : (kq + 1) * 8, :])

            tiles = {}

            def front_R(b, units=None):
                """R = relu(v@DxT) in [128,1024] relu units split ACT/DVE."""
                t = tiles.get(b)
                if t is None or "R" not in t:
                    if b not in vprevs:
                        stage_gather(b)
                    U, vprevT = vprevs.pop(b)
                    R = big.tile([128, TT, N], mm_dt, tag="R")
                    rs = tiny.tile([128, TT, 4], F32, tag="rs")
                    tiles[b] = {"U": U, "R": R, "rs": rs, "_vp": vprevT}
                t = tiles[b]
                R, rs, vprevT = t["R"], t["rs"], t["_vp"]
                for unit in (units if units is not None else range(8)):
                    m, g = divmod(unit, 4)
                    if True:
                        pr = psX.tile([128, 1024], F32, tag="psx")
                        for h in range(2):
                            nq = 2 * g + h
                            for kd in range(DT):
                                nc.tensor.matmul(
                                    pr[:, h * 512 : (h + 1) * 512],
                                    vprevT[:, kd, m * 128 : (m + 1) * 128],
                                    dxt[:, kd, nq * 512 : (nq + 1) * 512],
                                    start=(kd == 0),
                                    stop=(kd == DT - 1),
                                )
                        dst = R[:, m, g * 1024 : (g + 1) * 1024]
                        if (unit % 8) in ACT_R:
                            nc.scalar.activation(
                                out=dst, in_=pr[:], func=AF.Relu,
                                accum_out=rs[:, m, g : g + 1],
                            )
                        else:
                            # with accum_out: out = in0 op0 s1, accum = op1-reduce(out)
                            nc.vector.tensor_scalar(
                                out=dst, in0=pr[:], scalar1=0.0, scalar2=None,
                                op0=OP.max, op1=OP.add,
                                accum_out=rs[:, m, g : g + 1],
                            )

            def front_Ga(b):
                """q_t = (sum r_t + 0.97)/100 on gpsimd; transpose to a row."""
                t = tiles[b]
                rs = t.pop("rs")
                q16 = tiny.tile([128, TT], F16, tag="q16")
                for m in range(TT):
                    rsum = tiny.tile([128, 1], F32, tag="rsum")
                    nc.vector.tensor_reduce(
                        out=rsum[:], in_=rs[:, m, :], axis=mybir.AxisListType.X,
                        op=OP.add,
                    )
                    nc.vector.tensor_scalar(
                        out=q16[:, m : m + 1], in0=rsum[:],
                        scalar1=svb[:, m : m + 1], scalar2=0.01,
                        op0=OP.add, op1=OP.mult,
                    )
                pq = psS.tile([TT, 128], F16, tag="pss")
                nc.tensor.transpose(out=pq[:], in_=q16[:], identity=identh[:])
                qrow = tiny.tile([TT, 128], F16, tag="qrow")
                nc.scalar.copy(out=qrow[:], in_=pq[:])
                t["qrow"] = qrow

            def front_Gb(b):
                """Prefix products P (gpsimd scan) + shifted P_{s-1}."""
                t = tiles[b]
                qrow = t.pop("qrow")
                pad = scratch.tile([128, 128], F16, tag="pm1pad")
                with nc.allow_low_precision(
                    reason="scan state is fp32; f16 is storage only and the "
                    "banded G uses ratios where scan error cancels"
                ):
                    nc.vector.tensor_tensor_scan(
                        out=pad[0:2, :], data0=qrow[:], data1=qrow[:],
                        initial=1.0, op0=OP.mult, op1=OP.bypass,
                    )
                    # rows 32:34 = P_{s-1}: shifted copy of the scan output
                    # (exact; base partition must be a multiple of 32)
                    nc.gpsimd.tensor_copy(out=pad[32:34, 1:128],
                                          in_=pad[0:2, 0:127])
                    nc.gpsimd.memset(pad[32:34, 0:1], 1.0)
                pb = psS.tile([128, 128], F16, tag="pss")
                nc.tensor.transpose(out=pb[:], in_=pad[:], identity=identh[:])
                Pq4 = tiny.tile([128, 4], F32, tag="Pq4")
                nc.scalar.copy(out=Pq4[:, 0:2], in_=pb[:, 0:2])
                nc.scalar.copy(out=Pq4[:, 2:4], in_=pb[:, 32:34])
                t["Pq4"] = Pq4

            def front_Gc(b):
                """1/P row factors -> banded GT (2^8 scale)."""
                t = tiles[b]
                Pq4 = t.pop("Pq4")
                # 1/P on partitions (fast), then back to a row via transpose
                rPq = tiny.tile([128, TT], F16, tag="rPq")
                with nc.allow_low_precision(reason="1/P column factors, f16"):
                    nc.vector.reciprocal(out=rPq[:], in_=Pq4[:, 0:2])
                prt = psS.tile([TT, 128], F16, tag="pss")
                nc.tensor.transpose(out=prt[:], in_=rPq[:], identity=identh[:])
                rProw = tiny.tile([TT, 128], F16, tag="rProw")
                nc.scalar.copy(out=rProw[:], in_=prt[:])
                ptb = psS.tile([128, 1], F32, tag="pss")
                nc.tensor.matmul(ptb[:], esel[:, 0, :], rProw[:, 127:128],
                                 start=True, stop=True)
                # ucross = P_{s-1}(tile0) / Ptot0
                ucross = tiny.tile([128, 1], F32, tag="ucross")
                nc.scalar.activation(out=ucross[:], in_=ptb[:], func=AF.Copy,
                                     scale=Pq4[:, 2:3])
                GT = mid.tile([128, TT, T], mm_dt, tag="GT")
                for tau in range(TT):
                    pw = psS.tile([128, 128], F32, tag="pss")
                    nc.tensor.matmul(pw[:], esel[:, tau, :], rProw[:, :],
                                     start=True, stop=True)
                    for m in range(TT):
                        if tau == 0 and m == 1:
                            continue
                        rowf = (ucross[:] if (tau == 1 and m == 0)
                                else Pq4[:, 2 + m : 3 + m])
                        nc.vector.scalar_tensor_tensor(
                            out=GT[:, m, tau * 128 : (tau + 1) * 128],
                            in0=gexp[:, m, tau * 128 : (tau + 1) * 128],
                            scalar=rowf, in1=pw[:],
                            op0=OP.mult, op1=OP.mult,
                        )
                t["GT"] = GT

            def front_G(b):
                front_Ga(b)
                front_Gb(b)
                front_Gc(b)

            def stage_xt(b, groups):
                """X^T = R^T @ G^T (2^8 scale); s-tile 1 only feeds t >= 128."""
                t = tiles[b]
                if "XT" not in t:
                    t["XT"] = big.tile([128, NT, T], mm_dt, tag="XT", name="XT")
                XT, R, GT = t["XT"], t["R"], t["GT"]
                for ni in groups:
                    px = psX.tile([128, 4, 256], F32, tag="psx")
                    for h in range(4):
                        nt = 4 * ni + h
                        # banded X^T accumulation in three clean ranges so the
                        # psum zero-region state never mixes within one write
                        nc.tensor.matmul(
                            px[:, h, 0:144],
                            R[:, 0, nt * 128 : (nt + 1) * 128], GT[:, 0, 0:144],
                            start=True, stop=False, skip_group_check=True,
                        )
                        nc.tensor.matmul(
                            px[:, h, 128:144],
                            R[:, 1, nt * 128 : (nt + 1) * 128], GT[:, 1, 128:144],
                            start=False, stop=False, skip_group_check=True,
                        )
                        nc.tensor.matmul(
                            px[:, h, 144:256],
                            R[:, 1, nt * 128 : (nt + 1) * 128], GT[:, 1, 144:256],
                            start=False, stop=True, skip_group_check=True,
                        )
                    # two half-tile casts: scores can start consuming the
                    # first half ~0.6us earlier than a single 1024-el cast
                    for hh in range(2):
                        dst = XT[:, 4 * ni + 2 * hh : 4 * ni + 2 * hh + 2, :]
                        srcp = px[:, 2 * hh : 2 * hh + 2, :]
                        if (2 * ni + hh) % 2 == (0 if ni in ACT_XT else 1):
                            nc.scalar.copy(out=dst, in_=srcp)
                        else:
                            nc.vector.tensor_copy(out=dst, in_=srcp)

            def back_scores(b):
                """AT[s,t] = decay(t-s) * (x_s . x_t) (2^16 scale, f16)."""
                t = tiles[b]
                XT = t["XT"]
                AT = mid.tile([128, TT, T], F16, tag="AT")
                t["AT"] = AT
                for st in range(TT):
                    lo = st * 128
                    psc = psA.tile([128, T - lo], F32, tag="psa")
                    for k in range(NT):
                        nc.tensor.matmul(
                            psc[:], XT[:, k, lo : lo + 128], XT[:, k, lo:],
                            start=(k == 0), stop=(k == NT - 1),
                        )
                    nc.vector.tensor_tensor(
                        out=AT[:, st, lo:], in0=psc[:], in1=decayT[:, st, lo:],
                        op=OP.mult,
                    )

            def back_pa(b, tt):
                """a* = sum_s AT[s,t] u_s; ynorm = LN(a*) -> ynormT (f16)."""
                t = tiles[b]
                AT, U = t["AT"], t["U"]
                if "ynormT" not in t:
                    t["ynormT"] = mid.tile([128, DT, T], mm_dt, tag="ynormT", name="ynormT")
                ynormT = t["ynormT"]
                pa = psA.tile([128, D], F32, tag="psa")
                for k in range(tt + 1):
                    nc.tensor.matmul(
                        pa[:], AT[:, k, tt * 128 : (tt + 1) * 128], U[:, k, :],
                        start=(k == 0), stop=(k == tt),
                    )
                st6 = tiny.tile([128, 6], F32, tag="ln_st6")
                mv = tiny.tile([128, 2], F32, tag="ln_mv")
                nc.vector.bn_stats(out=st6[:], in_=pa[:])
                nc.vector.bn_aggr(out=mv[:], in_=st6[:])
                sa = tiny.tile([128, 1], F32, tag="ln_s")
                nc.scalar.activation(out=sa[:], in_=mv[:, 1:2], func=AF.Sqrt,
                                     scale=DDOF)
                sae = tiny.tile([128, 1], F32, tag="ln_se")
                nc.vector.tensor_scalar(out=sae[:], in0=sa[:], scalar1=EPS_A,
                                        scalar2=None, op0=OP.add)
                rstd = tiny.tile([128, 1], F32, tag="ln_r")
                nc.vector.reciprocal(out=rstd[:], in_=sae[:])
                ynorm = scratch.tile([128, D], F16, tag="ynorm")
                nc.vector.tensor_scalar(
                    out=ynorm[:], in0=pa[:], scalar1=mv[:, 0:1],
                    scalar2=rstd[:], op0=OP.subtract, op1=OP.mult,
                )
                t[("ynorm", tt)] = ynorm

            def back_ynt(b, tt):
                """ynorm -> ynormT transposes (emitted late so these PE-queue
                entries sit behind filler matmuls, not ahead of them)."""
                t = tiles[b]
                ynorm = t.pop(("ynorm", tt))
                ynormT = t["ynormT"]
                for kd in range(DT):
                    pt = psS.tile([128, 128], F16, tag="pss")
                    nc.tensor.transpose(
                        out=pt[:], in_=ynorm[:, kd * 128 : (kd + 1) * 128],
                        identity=identh[:],
                    )
                    nc.scalar.copy(
                        out=ynormT[:, kd, tt * 128 : (tt + 1) * 128], in_=pt[:]
                    )

            def back_y(b):
                """x~ = relu(LN(a*)@DyT) * x, fused relu-mult (f16, 2^8)."""
                t = tiles[b]
                XT, ynormT = t["XT"], t["ynormT"]
                for g in range(8):
                    py = psX.tile([128, 4, 256], F32, tag="psx")
                    for h in range(4):
                        nt = 4 * g + h
                        for kd in range(DT):
                            nc.tensor.matmul(
                                py[:, h, :],
                                dyt[:, kd, nt * 128 : (nt + 1) * 128],
                                ynormT[:, kd, :],
                                start=(kd == 0), stop=(kd == DT - 1),
                            )
                    xv = XT[:, 4 * g : 4 * g + 4, :]
                    if g in ACT_Y:
                        yr = scratch.tile([128, 4, 256], F16, tag="yrelu")
                        nc.scalar.activation(out=yr[:], in_=py[:], func=AF.Relu)
                        eng = nc.gpsimd if g in POOL_Y else nc.vector
                        eng.tensor_tensor(out=xv, in0=yr[:], in1=xv, op=OP.mult)
                    else:
                        # x >= 0 so relu(y)*x = max(y,0)*x in one DVE pass
                        nc.vector.scalar_tensor_tensor(
                            out=xv, in0=py[:], scalar=0.0, in1=xv,
                            op0=OP.max, op1=OP.mult,
                        )

            def back_v(b, tt):
                """v* = LN(x~ @ E^T) -> out rows tt*128..(tt+1)*128."""
                t = tiles[b]
                XT = t["XT"]
                pv = psA.tile([128, D], F32, tag="psa")
                for k in range(NT):
                    nc.tensor.matmul(
                        pv[:], XT[:, k, tt * 128 : (tt + 1) * 128], et[:, k, :],
                        start=(k == 0), stop=(k == NT - 1),
                    )
                st6 = tiny.tile([128, 6], F32, tag="ln_st6")
                mv = tiny.tile([128, 2], F32, tag="ln_mv")
                nc.vector.bn_stats(out=st6[:], in_=pv[:])
                nc.vector.bn_aggr(out=mv[:], in_=st6[:])
                sv = tiny.tile([128, 1], F32, tag="ln_s")
                nc.scalar.activation(out=sv[:], in_=mv[:, 1:2], func=AF.Sqrt,
                                     scale=DDOF)
                sve = tiny.tile([128, 1], F32, tag="ln_se")
                nc.vector.tensor_scalar(out=sve[:], in0=sv[:], scalar1=EPS_V,
                                        scalar2=None, op0=OP.add)
                rstd = tiny.tile([128, 1], F32, tag="ln_r")
                nc.vector.reciprocal(out=rstd[:], in_=sve[:])
                vstar = scratch.tile([128, D], F32, tag="vstar")
                nc.vector.tensor_scalar(
                    out=vstar[:], in0=pv[:], scalar1=mv[:, 0:1],
                    scalar2=rstd[:], op0=OP.subtract, op1=OP.mult,
                )
                nc.sync.dma_start(
                    out=out_d[b, tt * 128 : (tt + 1) * 128, :], in_=vstar[:]
                )

            # ---- software-pipelined emission: the next-next batch's R
            # matmuls and G chain fill the PE while the current batch's
            # casts / LN chains / masks run on ACT+DVE; the G chain's PE
            # hops are spaced behind big matmul batches so their cross-
            # engine deps resolve before the PE reaches them. ----
            front_R(0)
            emit_late_consts()
            front_G(0)
            front_R(1)
            stage_xt(0, range(8))
            front_G(1)
            for b in range(BL):
                back_scores(b)
                if b + 2 < BL:
                    front_R(b + 2, range(0, 4))
                back_pa(b, 0)
                if b + 2 < BL:
                    front_R(b + 2, range(4, 8))
                back_pa(b, 1)
                if b >= 1:
                    back_v(b - 1, 1)      # deferred: fills this pa->y hole
                if b + 1 < BL:
                    stage_xt(b + 1, range(0, 4))
                back_ynt(b, 0)
                back_ynt(b, 1)
                back_y(b)
                if b + 2 < BL:
                    front_Ga(b + 2)
                if b + 1 < BL:
                    stage_xt(b + 1, range(4, 6))
                back_v(b, 0)
                if b + 2 < BL:
                    front_Gb(b + 2)
                if b + 1 < BL:
                    stage_xt(b + 1, range(6, 8))
                if b == BL - 1:
                    back_v(b, 1)
                if b + 2 < BL:
                    front_Gc(b + 2)
                if b >= 1:
                    tiles[b - 1].clear()

    nc.compile()
    return nc


_NC_CACHE = {}


def _get_nc(mm_dt=MM_DT):
    key = str(mm_dt)
    if key not in _NC_CACHE:
        _NC_CACHE[key] = build_nc(mm_dt)
    return _NC_CACHE[key]


def make_in_maps(idx, token_emb, E, Dx, Dy, mm_dt=MM_DT):
    wdt = mybir.dt.np(mm_dt)
    idx = np.asarray(idx).astype(np.int64)
    token_emb32 = np.asarray(token_emb, np.float32)
    temb = token_emb32.astype(np.float16)
    dxt = np.ascontiguousarray(np.asarray(Dx, np.float32).T.astype(wdt))
    dyt = np.ascontiguousarray(np.asarray(Dy, np.float32).T.astype(wdt))
    et = np.ascontiguousarray(np.asarray(E, np.float32).T.astype(wdt))
    consts = _host_consts()
    identh = np.ascontiguousarray(np.eye(128, dtype=np.float16))
    esel = np.zeros((2, TT, 128), np.float16)
    for j in range(TT):
        esel[j, j, :] = 1.0
    shared = {"dxt": dxt, "dyt": dyt, "et": et,
              "identh": identh,
              "esel": np.ascontiguousarray(esel), **consts}
    in_maps = []
    for c in range(NCORES):
        m = dict(shared)
        vfull = temb[idx[c * BL : (c + 1) * BL]]            # (BL, T, D) f16
        v32 = token_emb32[idx[c * BL : (c + 1) * BL]]
        mu = v32.mean(axis=-1, keepdims=True)
        sd = v32.std(axis=-1, keepdims=True, ddof=1)
        u16 = ((v32 - mu) / (sd + EPS)).astype(np.float16)
        m["uln"] = np.ascontiguousarray(
            u16.reshape(BL, TT, 128, D).transpose(0, 2, 1, 3)
        )
        m["vprevT"] = np.ascontiguousarray(
            vfull.transpose(0, 2, 1).reshape(BL, DT, 128, T).transpose(0, 2, 1, 3)
        )
        in_maps.append(m)
    return in_maps


def kernel(idx, token_emb, E, Dx, Dy):
    nc = _get_nc()
    in_maps = make_in_maps(idx, token_emb, E, Dx, Dy)
    res = run_bass_kernel_spmd(nc, in_maps, core_ids=list(range(NCORES)))
    out = np.concatenate([r["out"] for r in res.results], axis=0)
    return out


# revision 32
# speedup vs baseline: 1.0265x; 1.0265x over previous
"""Trainium2 Bass kernel for the BDH fast-weight recurrent network.

Problem (see reference): for each batch element, a T=256-step recurrence with
  x_t   = L1norm(0.97*x_{t-1} + relu(v_t @ Dx^T))          (v_t = token_emb[idx_t])
  a*_t  = rho_{t-1} x_t ;  rho_t = 0.97*(rho_{t-1} + LN(v_t) x_t^T)
  y_t   = relu(LN(a*_t) @ Dy^T) * relu(x_t)
  out_t = LN(y_t @ E^T)

The kernel restructures this into feed-forward matmuls:
 - rho never materializes: a*_t = sum_{s<t} 0.97^{t-s} (x_s . x_t) LN(v_s)
   (decayed linear attention over the x sequence).
 - the x recurrence is linear given the per-step L1 scales S_t; since S_t ~ 100
   and eps=1e-6, S_t = sum(r_t) + 0.97 exactly in fp32, so X = G @ R with
   G[t,s] = 0.97^{t-s} / prod_{j=s..t} S_j.  G factors as
   gexp[s,t] * P_{s-1} / P_t with P_t = prod_{j<=t} (S_j/100) (range ~1, fp32
   safe) and gexp = host-precomputed exp part.  P is a prefix product done
   with a DVE scan; P_{s-1} comes from a shifted copy of the scan output (no
   reciprocal), 1/P returns to row layout via a partition-side reciprocal +
   PE transpose (fast) instead of a 128-element row reciprocal (940ns).
 - X carries a constant 2^8 factor (from the fp16-range shift in gexp).
 - x_t >= 0 so relu(x_t) = x_t and relu(y)*x = relu(y*x): the y-relu and the
   x multiply fuse into one DVE scalar_tensor_tensor pass (max then mult).
 - layernorms divide by (std + eps*2^k) directly, matching reference eps
   semantics exactly (no epsv folding).

Engine balance: the three 1M-element PSUM->SBUF passes per batch (R relu,
X^T cast, y-relu-mult) are split between the Scalar(ACT) and DVE engines;
a couple of SBUF-only multiplies ride on GpSimd.  Emission interleaves the
next batch's R matmuls and G chain into the current batch's serial LN/cast
regions so the PE queue never drains.

Sharding: data-parallel over batch, 4 sequences per NeuronCore x 8 cores,
no cross-core communication.
"""

import sys

if "/opt/trn_rl_repo" not in sys.path:
    sys.path.insert(0, "/opt/trn_rl_repo")

import numpy as np

import concourse.bass as bass
import concourse.bacc as bacc
import concourse.tile as tile
from concourse import mybir
from concourse.bass_utils import run_bass_kernel_spmd

AF = mybir.ActivationFunctionType
OP = mybir.AluOpType

N, D, V = 4096, 256, 32000
B, T = 32, 256
BL = 4              # batch per core
NCORES = 8
XD = 0.97           # x decay
UD = 0.97           # rho decay
EPS = 1e-6
MU = float(np.log(100.0))
LNXD = float(np.log(XD))

F32 = mybir.dt.float32
F16 = mybir.dt.float16
MODE = "f16"
MODE_DT = {"f32": mybir.dt.float32, "f32r": mybir.dt.float32r,
           "f16": mybir.dt.float16, "fp8": mybir.dt.float16}
MM_DT = MODE_DT[MODE]
GT_LOG_SCALE = 8.0 * float(np.log(2.0))   # store GT * 2^8 (fp16 underflow guard)
EPS_A = EPS * float(2.0 ** 16)            # a* psum carries 2^16 (= 2*2^8)
EPS_V = EPS * float(2.0 ** 8)             # v* psum carries 2^8 (x~ has 2^8)

NT = N // 128       # 32 n tiles
TT = T // 128       # 2 t tiles
DT = D // 128       # 2 d tiles
DDOF = float(D) / (D - 1)

# engine split knobs (chunk index -> ACT engine; rest DVE)
import os as _os

def _knob(name, default):
    v = _os.environ.get(name)
    if v is None:
        return default
    return tuple(int(x) for x in v.split(",") if x != "")

ACT_R = _knob("K_ACT_R", (0, 2, 4, 6))    # of 8 R-relu units [128,1024]/batch
ACT_XT = _knob("K_ACT_XT", (0, 2, 4, 6))  # of 8 XT-cast units per batch
ACT_Y = _knob("K_ACT_Y", (1, 3, 5))       # of 8 y units: ACT relu + mult; rest DVE STT
POOL_Y = _knob("K_POOL_Y", (3,))          # subset of ACT_Y whose mult goes to gpsimd


def _host_consts():
    """Constant tensors shipped to every core (computed in float64, cast f32)."""
    si = np.arange(T, dtype=np.float64)[:, None]
    ti = np.arange(T, dtype=np.float64)[None, :]
    k = ti - si
    kconst = np.where(k >= 0, k * LNXD - (k + 1) * MU + GT_LOG_SCALE, -np.inf)
    gexp = np.exp(kconst).astype(np.float32)          # banded: underflow -> 0
    gexp = gexp.reshape(TT, 128, T).transpose(1, 0, 2)
    decayT = np.where(k > 0, UD ** np.maximum(k, 0.0), 0.0)
    decayT = decayT.astype(np.float32).reshape(TT, 128, T).transpose(1, 0, 2)
    svb = np.full((T,), XD, np.float32)
    svb[0] = 0.0
    svb = svb.reshape(TT, 128).T.copy()
    return {
        "gexp": np.ascontiguousarray(gexp),       # (128, TT, T)
        "decayT": np.ascontiguousarray(decayT),   # (128, TT, T)
        "svb": np.ascontiguousarray(svb),         # (128, TT)
    }


def build_nc(mm_dt=MM_DT, dbg=False, dbg_keys=None):
    nc = bacc.Bacc("TRN2", target_bir_lowering=False, debug=False)


    ud_d = nc.dram_tensor("uln", [BL, 128, TT, D], F16, kind="ExternalInput").ap()
    vptd = nc.dram_tensor("vprevT", [BL, 128, DT, T], F16, kind="ExternalInput").ap()
    dxt_d = nc.dram_tensor("dxt", [D, N], mm_dt, kind="ExternalInput").ap()
    dyt_d = nc.dram_tensor("dyt", [D, N], mm_dt, kind="ExternalInput").ap()
    et_d = nc.dram_tensor("et", [N, D], mm_dt, kind="ExternalInput").ap()
    gexp_d = nc.dram_tensor("gexp", [128, TT, T], F32, kind="ExternalInput").ap()
    decayT_d = nc.dram_tensor("decayT", [128, TT, T], F32, kind="ExternalInput").ap()
    svb_d = nc.dram_tensor("svb", [128, TT], F32, kind="ExternalInput").ap()
    identh_d = nc.dram_tensor("identh", [128, 128], F16, kind="ExternalInput").ap()
    esel_d = nc.dram_tensor("esel", [2, TT, 128], F16, kind="ExternalInput").ap()
    out_d = nc.dram_tensor("out", [BL, T, D], F32, kind="ExternalOutput").ap()

    with tile.TileContext(nc) as tc:
        with (
            tc.tile_pool(name="consts", bufs=1) as consts,
            tc.tile_pool(name="big", bufs=2) as big,
            tc.tile_pool(name="mid", bufs=2) as mid,
            tc.tile_pool(name="tiny", bufs=10) as tiny,
            tc.tile_pool(name="scratch", bufs=6) as scratch,
            tc.tile_pool(name="vpool", bufs=3) as vpool,
            tc.tile_pool(name="psX", bufs=2, space="PSUM") as psX,
            tc.tile_pool(name="psA", bufs=3, space="PSUM") as psA,
            tc.tile_pool(name="psS", bufs=1, space="PSUM") as psS,
        ):
            # ---- embedding rows are gathered host-side (idx known there) ----
            vprevs = {}

            def stage_gather(b, eng=None):
                U = vpool.tile([128, TT, D], F16, tag="U")
                vprevT = vpool.tile([128, DT, T], F16, tag="vprevT")
                vprevs[b] = (U, vprevT)
                e = eng if eng is not None else nc.sync
                e.dma_start(out=vprevT[:], in_=vptd[b])
                e.dma_start(out=U[:], in_=ud_d[b])

            dxt = consts.tile([128, DT, N], mm_dt)
            dxt_src = dxt_d.rearrange("(k p) n -> p k n", p=128)
            # critical prologue transfers: gather(0) dispatches on the (idle)
            # ACT queue in parallel with the first dxt chunks on SP
            stage_gather(0, eng=nc.scalar)
            for kd in range(DT):
                nc.sync.dma_start(out=dxt[:, kd, 0:512],
                                  in_=dxt_src[:, kd, 0:512])
            for kd in range(DT):
                nc.sync.dma_start(out=dxt[:, kd, 512:1024],
                                  in_=dxt_src[:, kd, 512:1024])
            for q in range(1, 4):
                for kd in range(DT):
                    nc.sync.dma_start(
                        out=dxt[:, kd, q * 1024 : (q + 1) * 1024],
                        in_=dxt_src[:, kd, q * 1024 : (q + 1) * 1024])
            stage_gather(1)
            identh = consts.tile([128, 128], F16)
            nc.sync.dma_start(out=identh[:], in_=identh_d[:])
            svb = consts.tile([128, TT], F32)
            nc.sync.dma_start(out=svb[:], in_=svb_d[:])
            gexp = consts.tile([128, TT, T], F32)
            nc.sync.dma_start(out=gexp[:], in_=gexp_d[:])
            esel = consts.tile([2, TT, 128], F16)
            nc.sync.dma_start(out=esel[:], in_=esel_d[:])

            decayT = consts.tile([128, TT, T], F32)
            dyt = consts.tile([128, DT, N], mm_dt)
            et = consts.tile([128, NT, D], mm_dt)

            def emit_late_consts():
                nc.sync.dma_start(out=decayT[:], in_=decayT_d[:])
                dyt_src = dyt_d.rearrange("(k p) n -> p k n", p=128)
                for kd in range(DT):
                    nc.sync.dma_start(out=dyt[:, kd, :], in_=dyt_src[:, kd, :])
                et_src = et_d.rearrange("(k p) d -> p k d", p=128)
                for kq in range(4):
                    nc.sync.dma_start(out=et[:, kq * 8 : (kq + 1) * 8, :],
                                      in_=et_src[:, kq * 8 : (kq + 1) * 8, :])

            tiles = {}

            def front_R(b, units=None):
                """R = relu(v@DxT) in [128,1024] relu units split ACT/DVE."""
                t = tiles.get(b)
                if t is None or "R" not in t:
                    if b not in vprevs:
                        stage_gather(b)
                    U, vprevT = vprevs.pop(b)
                    R = big.tile([128, TT, N], mm_dt, tag="R")
                    rs = tiny.tile([128, TT, 4], F32, tag="rs")
                    tiles[b] = {"U": U, "R": R, "rs": rs, "_vp": vprevT}
                t = tiles[b]
                R, rs, vprevT = t["R"], t["rs"], t["_vp"]
                for unit in (units if units is not None else range(8)):
                    m, g = divmod(unit, 4)
                    if True:
                        pr = psX.tile([128, 1024], F32, tag="psx")
                        for h in range(2):
                            nq = 2 * g + h
                            for kd in range(DT):
                                nc.tensor.matmul(
                                    pr[:, h * 512 : (h + 1) * 512],
                                    vprevT[:, kd, m * 128 : (m + 1) * 128],
                                    dxt[:, kd, nq * 512 : (nq + 1) * 512],
                                    start=(kd == 0),
                                    stop=(kd == DT - 1),
                                )
                        dst = R[:, m, g * 1024 : (g + 1) * 1024]
                        if (unit % 8) in ACT_R:
                            nc.scalar.activation(
                                out=dst, in_=pr[:], func=AF.Relu,
                                accum_out=rs[:, m, g : g + 1],
                            )
                        else:
                            # with accum_out: out = in0 op0 s1, accum = op1-reduce(out)
                            nc.vector.tensor_scalar(
                                out=dst, in0=pr[:], scalar1=0.0, scalar2=None,
                                op0=OP.max, op1=OP.add,
                                accum_out=rs[:, m, g : g + 1],
                            )

            def front_Ga(b):
                """q_t = (sum r_t + 0.97)/100 on gpsimd; transpose to a row."""
                t = tiles[b]
                rs = t.pop("rs")
                q16 = tiny.tile([128, TT], F16, tag="q16")
                for m in range(TT):
                    rsum = tiny.tile([128, 1], F32, tag="rsum")
                    nc.vector.tensor_reduce(
                        out=rsum[:], in_=rs[:, m, :], axis=mybir.AxisListType.X,
                        op=OP.add,
                    )
                    nc.vector.tensor_scalar(
                        out=q16[:, m : m + 1], in0=rsum[:],
                        scalar1=svb[:, m : m + 1], scalar2=0.01,
                        op0=OP.add, op1=OP.mult,
                    )
                pq = psS.tile([TT, 128], F16, tag="pss")
                nc.tensor.transpose(out=pq[:], in_=q16[:], identity=identh[:])
                qrow = tiny.tile([TT, 128], F16, tag="qrow")
                nc.scalar.copy(out=qrow[:], in_=pq[:])
                t["qrow"] = qrow

            def front_Gb(b):
                """Prefix products P (gpsimd scan) + shifted P_{s-1}."""
                t = tiles[b]
                qrow = t.pop("qrow")
                pad = scratch.tile([128, 128], F16, tag="pm1pad")
                with nc.allow_low_precision(
                    reason="scan state is fp32; f16 is storage only and the "
                    "banded G uses ratios where scan error cancels"
                ):
                    nc.vector.tensor_tensor_scan(
                        out=pad[0:2, :], data0=qrow[:], data1=qrow[:],
                        initial=1.0, op0=OP.mult, op1=OP.bypass,
                    )
                    # rows 32:34 = P_{s-1}: shifted copy of the scan output
                    # (exact; base partition must be a multiple of 32)
                    nc.gpsimd.tensor_copy(out=pad[32:34, 1:128],
                                          in_=pad[0:2, 0:127])
                    nc.gpsimd.memset(pad[32:34, 0:1], 1.0)
                pb = psS.tile([128, 128], F16, tag="pss")
                nc.tensor.transpose(out=pb[:], in_=pad[:], identity=identh[:])
                Pq4 = tiny.tile([128, 4], F32, tag="Pq4")
                nc.scalar.copy(out=Pq4[:, 0:2], in_=pb[:, 0:2])
                nc.scalar.copy(out=Pq4[:, 2:4], in_=pb[:, 32:34])
                t["Pq4"] = Pq4

            def front_Gc(b):
                """1/P row factors -> banded GT (2^8 scale)."""
                t = tiles[b]
                Pq4 = t.pop("Pq4")
                # 1/P on partitions (fast), then back to a row via transpose
                rPq = tiny.tile([128, TT], F16, tag="rPq")
                with nc.allow_low_precision(reason="1/P column factors, f16"):
                    nc.vector.reciprocal(out=rPq[:], in_=Pq4[:, 0:2])
                prt = psS.tile([TT, 128], F16, tag="pss")
                nc.tensor.transpose(out=prt[:], in_=rPq[:], identity=identh[:])
                rProw = tiny.tile([TT, 128], F16, tag="rProw")
                nc.scalar.copy(out=rProw[:], in_=prt[:])
                ptb = psS.tile([128, 1], F32, tag="pss")
                nc.tensor.matmul(ptb[:], esel[:, 0, :], rProw[:, 127:128],
                                 start=True, stop=True)
                # ucross = P_{s-1}(tile0) / Ptot0
                ucross = tiny.tile([128, 1], F32, tag="ucross")
                nc.scalar.activation(out=ucross[:], in_=ptb[:], func=AF.Copy,
                                     scale=Pq4[:, 2:3])
                GT = mid.tile([128, TT, T], mm_dt, tag="GT")
                for tau in range(TT):
                    pw = psS.tile([128, 128], F32, tag="pss")
                    nc.tensor.matmul(pw[:], esel[:, tau, :], rProw[:, :],
                                     start=True, stop=True)
                    for m in range(TT):
                        if tau == 0 and m == 1:
                            continue
                        rowf = (ucross[:] if (tau == 1 and m == 0)
                                else Pq4[:, 2 + m : 3 + m])
                        nc.vector.scalar_tensor_tensor(
                            out=GT[:, m, tau * 128 : (tau + 1) * 128],
                            in0=gexp[:, m, tau * 128 : (tau + 1) * 128],
                            scalar=rowf, in1=pw[:],
                            op0=OP.mult, op1=OP.mult,
                        )
                t["GT"] = GT

            def front_G(b):
                front_Ga(b)
                front_Gb(b)
                front_Gc(b)

            def stage_xt(b, groups):
                """X^T = R^T @ G^T (2^8 scale); s-tile 1 only feeds t >= 128."""
                t = tiles[b]
                if "XT" not in t:
                    t["XT"] = big.tile([128, NT, T], mm_dt, tag="XT", name="XT")
                XT, R, GT = t["XT"], t["R"], t["GT"]
                for ni in groups:
                    px = psX.tile([128, 4, 256], F32, tag="psx")
                    for h in range(4):
                        nt = 4 * ni + h
                        # banded X^T accumulation in three clean ranges so the
                        # psum zero-region state never mixes within one write
                        nc.tensor.matmul(
                            px[:, h, 0:144],
                            R[:, 0, nt * 128 : (nt + 1) * 128], GT[:, 0, 0:144],
                            start=True, stop=False, skip_group_check=True,
                        )
                        nc.tensor.matmul(
                            px[:, h, 128:144],
                            R[:, 1, nt * 128 : (nt + 1) * 128], GT[:, 1, 128:144],
                            start=False, stop=False, skip_group_check=True,
                        )
                        nc.tensor.matmul(
                            px[:, h, 144:256],
                            R[:, 1, nt * 128 : (nt + 1) * 128], GT[:, 1, 144:256],
                            start=False, stop=True, skip_group_check=True,
                        )
                    dst = XT[:, 4 * ni : 4 * ni + 4, :]
                    if ni in ACT_XT:
                        nc.scalar.copy(out=dst, in_=px[:])
                    else:
                        nc.vector.tensor_copy(out=dst, in_=px[:])

            def back_scores(b):
                """AT[s,t] = decay(t-s) * (x_s . x_t) (2^16 scale, f16)."""
                t = tiles[b]
                XT = t["XT"]
                AT = mid.tile([128, TT, T], F16, tag="AT")
                t["AT"] = AT
                for st in range(TT):
                    lo = st * 128
                    psc = psA.tile([128, T - lo], F32, tag="psa")
                    for k in range(NT):
                        nc.tensor.matmul(
                            psc[:], XT[:, k, lo : lo + 128], XT[:, k, lo:],
                            start=(k == 0), stop=(k == NT - 1),
                        )
                    nc.vector.tensor_tensor(
                        out=AT[:, st, lo:], in0=psc[:], in1=decayT[:, st, lo:],
                        op=OP.mult,
                    )

            def back_pa(b, tt):
                """a* = sum_s AT[s,t] u_s; ynorm = LN(a*) -> ynormT (f16)."""
                t = tiles[b]
                AT, U = t["AT"], t["U"]
                if "ynormT" not in t:
                    t["ynormT"] = mid.tile([128, DT, T], mm_dt, tag="ynormT", name="ynormT")
                ynormT = t["ynormT"]
                pa = psA.tile([128, D], F32, tag="psa")
                for k in range(tt + 1):
                    nc.tensor.matmul(
                        pa[:], AT[:, k, tt * 128 : (tt + 1) * 128], U[:, k, :],
                        start=(k == 0), stop=(k == tt),
                    )
                st6 = tiny.tile([128, 6], F32, tag="ln_st6")
                mv = tiny.tile([128, 2], F32, tag="ln_mv")
                nc.vector.bn_stats(out=st6[:], in_=pa[:])
                nc.vector.bn_aggr(out=mv[:], in_=st6[:])
                sa = tiny.tile([128, 1], F32, tag="ln_s")
                nc.scalar.activation(out=sa[:], in_=mv[:, 1:2], func=AF.Sqrt,
                                     scale=DDOF)
                sae = tiny.tile([128, 1], F32, tag="ln_se")
                nc.vector.tensor_scalar(out=sae[:], in0=sa[:], scalar1=EPS_A,
                                        scalar2=None, op0=OP.add)
                rstd = tiny.tile([128, 1], F32, tag="ln_r")
                nc.vector.reciprocal(out=rstd[:], in_=sae[:])
                ynorm = scratch.tile([128, D], F16, tag="ynorm")
                nc.vector.tensor_scalar(
                    out=ynorm[:], in0=pa[:], scalar1=mv[:, 0:1],
                    scalar2=rstd[:], op0=OP.subtract, op1=OP.mult,
                )
                t[("ynorm", tt)] = ynorm

            def back_ynt(b, tt):
                """ynorm -> ynormT transposes (emitted late so these PE-queue
                entries sit behind filler matmuls, not ahead of them)."""
                t = tiles[b]
                ynorm = t.pop(("ynorm", tt))
                ynormT = t["ynormT"]
                for kd in range(DT):
                    pt = psS.tile([128, 128], F16, tag="pss")
                    nc.tensor.transpose(
                        out=pt[:], in_=ynorm[:, kd * 128 : (kd + 1) * 128],
                        identity=identh[:],
                    )
                    nc.scalar.copy(
                        out=ynormT[:, kd, tt * 128 : (tt + 1) * 128], in_=pt[:]
                    )

            def back_y(b):
                """x~ = relu(LN(a*)@DyT) * x, fused relu-mult (f16, 2^8)."""
                t = tiles[b]
                XT, ynormT = t["XT"], t["ynormT"]
                for g in range(8):
                    py = psX.tile([128, 4, 256], F32, tag="psx")
                    for h in range(4):
                        nt = 4 * g + h
                        for kd in range(DT):
                            nc.tensor.matmul(
                                py[:, h, :],
                                dyt[:, kd, nt * 128 : (nt + 1) * 128],
                                ynormT[:, kd, :],
                                start=(kd == 0), stop=(kd == DT - 1),
                            )
                    xv = XT[:, 4 * g : 4 * g + 4, :]
                    if g in ACT_Y:
                        yr = scratch.tile([128, 4, 256], F16, tag="yrelu")
                        nc.scalar.activation(out=yr[:], in_=py[:], func=AF.Relu)
                        eng = nc.gpsimd if g in POOL_Y else nc.vector
                        eng.tensor_tensor(out=xv, in0=yr[:], in1=xv, op=OP.mult)
                    else:
                        # x >= 0 so relu(y)*x = max(y,0)*x in one DVE pass
                        nc.vector.scalar_tensor_tensor(
                            out=xv, in0=py[:], scalar=0.0, in1=xv,
                            op0=OP.max, op1=OP.mult,
                        )

            def back_v(b, tt):
                """v* = LN(x~ @ E^T) -> out rows tt*128..(tt+1)*128."""
                t = tiles[b]
                XT = t["XT"]
                pv = psA.tile([128, D], F32, tag="psa")
                for k in range(NT):
                    nc.tensor.matmul(
                        pv[:], XT[:, k, tt * 128 : (tt + 1) * 128], et[:, k, :],
                        start=(k == 0), stop=(k == NT - 1),
                    )
                st6 = tiny.tile([128, 6], F32, tag="ln_st6")
                mv = tiny.tile([128, 2], F32, tag="ln_mv")
                nc.vector.bn_stats(out=st6[:], in_=pv[:])
                nc.vector.bn_aggr(out=mv[:], in_=st6[:])
                sv = tiny.tile([128, 1], F32, tag="ln_s")
                nc.scalar.activation(out=sv[:], in_=mv[:, 1:2], func=AF.Sqrt,
                                     scale=DDOF)
                sve = tiny.tile([128, 1], F32, tag="ln_se")
                nc.vector.tensor_scalar(out=sve[:], in0=sv[:], scalar1=EPS_V,
                                        scalar2=None, op0=OP.add)
                rstd = tiny.tile([128, 1], F32, tag="ln_r")
                nc.vector.reciprocal(out=rstd[:], in_=sve[:])
                vstar = scratch.tile([128, D], F32, tag="vstar")
                nc.vector.tensor_scalar(
                    out=vstar[:], in0=pv[:], scalar1=mv[:, 0:1],
                    scalar2=rstd[:], op0=OP.subtract, op1=OP.mult,
                )
                nc.sync.dma_start(
                    out=out_d[b, tt * 128 : (tt + 1) * 128, :], in_=vstar[:]
                )

            # ---- software-pipelined emission: the next-next batch's R
            # matmuls and G chain fill the PE while the current batch's
            # casts / LN chains / masks run on ACT+DVE; the G chain's PE
            # hops are spaced behind big matmul batches so their cross-
            # engine deps resolve before the PE reaches them. ----
            front_R(0)
            emit_late_consts()
            front_G(0)
            front_R(1)
            stage_xt(0, range(8))
            front_G(1)
            for b in range(BL):
                back_scores(b)
                if b + 2 < BL:
                    front_R(b + 2, range(0, 4))
                back_pa(b, 0)
                if b + 2 < BL:
                    front_R(b + 2, range(4, 8))
                back_pa(b, 1)
                if b >= 1:
                    back_v(b - 1, 1)      # deferred: fills this pa->y hole
                if b + 1 < BL:
                    stage_xt(b + 1, range(0, 4))
                back_ynt(b, 0)
                back_ynt(b, 1)
                back_y(b)
                if b + 2 < BL:
                    front_Ga(b + 2)
                if b + 1 < BL:
                    stage_xt(b + 1, range(4, 6))
                back_v(b, 0)
                if b + 2 < BL:
                    front_Gb(b + 2)
                if b + 1 < BL:
                    stage_xt(b + 1, range(6, 8))
                if b == BL - 1:
                    back_v(b, 1)
                if b + 2 < BL:
                    front_Gc(b + 2)
                if b >= 1:
                    tiles[b - 1].clear()

    nc.compile()
    return nc


_NC_CACHE = {}


def _get_nc(mm_dt=MM_DT):
    key = str(mm_dt)
    if key not in _NC_CACHE:
        _NC_CACHE[key] = build_nc(mm_dt)
    return _NC_CACHE[key]


def make_in_maps(idx, token_emb, E, Dx, Dy, mm_dt=MM_DT):
    wdt = mybir.dt.np(mm_dt)
    idx = np.asarray(idx).astype(np.int64)
    token_emb32 = np.asarray(token_emb, np.float32)
    temb = token_emb32.astype(np.float16)
    dxt = np.ascontiguousarray(np.asarray(Dx, np.float32).T.astype(wdt))
    dyt = np.ascontiguousarray(np.asarray(Dy, np.float32).T.astype(wdt))
    et = np.ascontiguousarray(np.asarray(E, np.float32).T.astype(wdt))
    consts = _host_consts()
    identh = np.ascontiguousarray(np.eye(128, dtype=np.float16))
    esel = np.zeros((2, TT, 128), np.float16)
    for j in range(TT):
        esel[j, j, :] = 1.0
    shared = {"dxt": dxt, "dyt": dyt, "et": et,
              "identh": identh,
              "esel": np.ascontiguousarray(esel), **consts}
    in_maps = []
    for c in range(NCORES):
        m = dict(shared)
        vfull = temb[idx[c * BL : (c + 1) * BL]]            # (BL, T, D) f16
        v32 = token_emb32[idx[c * BL : (c + 1) * BL]]
        mu = v32.mean(axis=-1, keepdims=True)
        sd = v32.std(axis=-1, keepdims=True, ddof=1)
        u16 = ((v32 - mu) / (sd + EPS)).astype(np.float16)
        m["uln"] = np.ascontiguousarray(
            u16.reshape(BL, TT, 128, D).transpose(0, 2, 1, 3)
        )
        m["vprevT"] = np.ascontiguousarray(
            vfull.transpose(0, 2, 1).reshape(BL, DT, 128, T).transpose(0, 2, 1, 3)
        )
        in_maps.append(m)
    return in_maps


def kernel(idx, token_emb, E, Dx, Dy):
    nc = _get_nc()
    in_maps = make_in_maps(idx, token_emb, E, Dx, Dy)
    res = run_bass_kernel_spmd(nc, in_maps, core_ids=list(range(NCORES)))
    out = np.concatenate([r["out"] for r in res.results], axis=0)
    return out
